# Initial kernel scaffold
#
"""Trainium2 Bass kernel for DigitCapsuleLayer dynamic routing.

Strategy: data-parallel over batch (32 per core x 8 cores). The routing is
computed in a fully factored form that never materializes u_hat
[B,1152,10,16]:

  q[b,c,m,i] = sum_g  cij[(g,m),c] * u[b,(g,m),i]      (PE, block-diag cij)
  s[b,c,o]   = sum_mi W[m,c,o,i]   * q[b,c,m,i]        (PE, after a DVE
                                                        32x32 block transpose
                                                        moves i to partitions)
  v = squash(s)                                        (PE ones-trick + DVE/ACT)
  p[b,c,m,i] = sum_o  W[m,c,o,i]   * v[b,c,o]          (PE, block-diag v)
  a[r,c]     = sum_bi u[b,r,i]/B   * p[b,c,m,i]        (PE)
  AllReduce(a) across 8 cores; b_ij += a; cij = softmax(b_ij)

Indices: r = g*32+m (g<36, m<32), m = 2t+m_sub, i = h*4+i4 (h<2, i4<4).
Partition layouts:  P1 rows = m_sub*64+g (rows 36..63/100..127 zero-padded),
q/p rows = i4*32+b (per half/blk), s/v rows = i4*16+o (4x replicated).
All index algebra validated against the jax reference in proto.py.
"""

import os
import sys
import numpy as np

sys.path.insert(0, "/opt/trn_rl_repo")
sys.path.insert(0, "/opt/trn_rl_repo/concourse")

NC_CORES = 8
BL = 32           # batch per core
G, M32, C, O, I = 36, 32, 10, 16, 8
T16 = 16
F32 = None        # set after mybir import


# ----------------------------------------------------------------- host prep
def _host_prep(u, W):
    """u [256,1152,8] f32, W [32,10,16,8] f32 -> per-core input maps."""
    u = np.ascontiguousarray(u, np.float32)
    W = np.ascontiguousarray(W, np.float32)

    # u_qp [core, 128, t, h, 128]; row p = m_sub*64+g ; col = i4*32+b
    u8 = u.reshape(NC_CORES, BL, G, T16, 2, 2, 4)   # [n, b, g, t, ms, h, i4]
    perm = u8.transpose(0, 2, 4, 3, 5, 6, 1)        # [n, g, ms, t, h, i4, b]
    u_qp = np.zeros((NC_CORES, 128, T16, 2, 128), np.float32)
    u_qp_v = u_qp.reshape(NC_CORES, 128, T16, 2, 4, 32)
    for ms in range(2):
        u_qp_v[:, ms * 64:ms * 64 + G] = perm[:, :, ms]

    # u_a2 [core, k, 128, 36, 32]: row i4*32+b, col (g, m); prescaled by 1/256
    ua = u.reshape(NC_CORES, BL, G, M32, 2, 4)      # [n, b, g, m, k, i4]
    u_a2 = np.ascontiguousarray(
        ua.transpose(0, 4, 5, 1, 2, 3), np.float32
    ).reshape(NC_CORES, 2, 128, G, M32) * np.float32(1.0 / 256.0)

    # w_s4 [k, 128, c, 128]: row i4*32+m, col (rep, half, o): half0 = W[m,c,o,i],
    # half1 = 0 pad so matmul M=128 lands v at partitions rep*32+o.
    wi = W.transpose(3, 0, 1, 2)                    # [i, m, c, o]
    w_s4 = np.zeros((2, 4, M32, C, 4, 2, O), np.float32)
    w_s4[:, :, :, :, :, 0, :] = wi.reshape(2, 4, M32, C, 1, O)
    w_s4 = np.ascontiguousarray(w_s4.reshape(2, 128, C, 128))

    # w_p2 [128, c, blk, m]: row i4*32+o (o<16; +16..31 zero) = W[m,c,o,blk*4+i4]
    wp = W.transpose(3, 2, 1, 0)                    # [i, o, c, m]
    w_p2 = np.zeros((4, 2, O, C, 2, M32), np.float32)
    w_p2[:, 0] = wp.reshape(2, 4, O, C, M32).transpose(1, 2, 3, 0, 4)
    w_p2 = np.ascontiguousarray(w_p2.reshape(128, C, 2, M32))

    # ones_bd [128, 128]: 16x16 ones blocks at (i4*32..+16, i4*32..+16)
    ones_bd = np.zeros((4, 2, O, 4, 2, O), np.float32)
    for j in range(4):
        ones_bd[j, 0, :, j, 0, :] = 1.0
    ones_bd = np.ascontiguousarray(ones_bd.reshape(128, 128))

    in_maps = []
    for c in range(NC_CORES):
        in_maps.append({
            "u_qp": u_qp[c],
            "u_a2": u_a2[c],
            "w_s4": w_s4,
            "w_p2": w_p2,
            "ones_bd": ones_bd,
        })
    return in_maps


# ------------------------------------------------------------- bass builder
def _build_nc():
    from contextlib import ExitStack
    import concourse.bacc as bacc
    import concourse.tile as tile
    from concourse import mybir

    f32 = mybir.dt.float32
    f32r = mybir.dt.float32r
    # per-matmul-family fp32r enable (bisection switches)
    RQ = RS = RM = RP = RA = False
    dt_q = f32r if RQ else f32
    dt_s = f32r if RS else f32
    dt_m = f32r if RM else f32
    dt_p = f32r if RP else f32
    dt_a = f32r if RA else f32
    nc = bacc.Bacc("TRN2", target_bir_lowering=False, debug=False,
                   num_devices=NC_CORES)

    u_qp_p = nc.dram_tensor("u_qp", [128, T16, 2, 128], dt_q, kind="ExternalInput")
    u_a2_p = nc.dram_tensor("u_a2", [2, 128, G, M32], dt_a, kind="ExternalInput")
    w_s4_p = nc.dram_tensor("w_s4", [2, 128, C, 128], dt_s, kind="ExternalInput")
    w_p2_p = nc.dram_tensor("w_p2", [128, C, 2, M32], dt_p, kind="ExternalInput")
    ones_p = nc.dram_tensor("ones_bd", [128, 128], dt_m, kind="ExternalInput")
    v_out_p = nc.dram_tensor("v_out", [16, C, BL], f32, kind="ExternalOutput")

    rg = [list(range(NC_CORES))]

    with tile.TileContext(nc) as tc, ExitStack() as ctx:
        sb = ctx.enter_context(tc.tile_pool(name="sb", bufs=1))
        ps = ctx.enter_context(tc.tile_pool(name="ps", bufs=1, space="PSUM"))
        dr = ctx.enter_context(tc.tile_pool(name="dr", bufs=1, space="DRAM"))

        # --- persistent SBUF tiles ---
        u_qp = sb.tile([128, T16, 2, 128], dt_q, tag="u_qp")
        u_a2 = [sb.tile([128, G, M32], dt_a, tag=f"u_a2_{k}", name=f"u_a2_{k}") for k in range(2)]
        w_s4 = [sb.tile([128, C, 128], dt_s, tag=f"w_s4_{k}", name=f"w_s4_{k}") for k in range(2)]
        w_p2 = sb.tile([128, C, 2, M32], dt_p, tag="w_p2")
        ones = sb.tile([128, 128], dt_m, tag="ones")
        b_ij = sb.tile([128, T16, C], f32, tag="b_ij")
        cij_bd = sb.tile([128, T16, 2, C], dt_q, tag="cij_bd")
        v_bd = sb.tile([128, C, 4, BL], dt_p, tag="v_bd")

        for tchunk in range(4):
            nc.sync.dma_start(out=u_qp[:, 4 * tchunk:4 * (tchunk + 1)],
                              in_=u_qp_p[:, 4 * tchunk:4 * (tchunk + 1)])
        for k in range(2):
            nc.sync.dma_start(out=u_a2[k][:], in_=u_a2_p[k])
            nc.sync.dma_start(out=w_s4[k][:], in_=w_s4_p[k])
        nc.sync.dma_start(out=w_p2[:], in_=w_p2_p[:])
        nc.sync.dma_start(out=ones[:], in_=ones_p[:])
        nc.vector.memset(b_ij[:], 0.0)
        # warm-up collective: absorbs first-AR setup/handshake off the
        # critical path while iter-0 compute proceeds
        warm_in = dr.tile([1, 20], f32, tag="warm_in", name="warm_in")
        warm_out = dr.tile([1, 20], f32, tag="warm_out", name="warm_out",
                           addr_space="Shared")
        nc.sync.dma_start(out=warm_in[:], in_=b_ij[0:1, 0:2, :])
        nc.gpsimd.collective_compute(
            "AllReduce", mybir.AluOpType.add, replica_groups=rg,
            ins=[warm_in[:].opt()], outs=[warm_out[:].opt()])
        nc.vector.memset(cij_bd[:].bitcast(f32), 0.0)
        nc.vector.memset(v_bd[:].bitcast(f32), 0.0)

        # --- PSUM tiles (persist across iterations) ---
        q_psum = [ps.tile([128, T16, 2, C], f32, tag=f"q_ps{h}", name=f"q_ps{h}") for h in range(2)]
        s_psum = ps.tile([128, C, BL], f32, tag="s_ps")
        mag_ps = ps.tile([128, C, BL], f32, tag="mag_ps")
        p_psum = [ps.tile([128, C, M32], f32, tag=f"p_ps{b}", name=f"p_ps{b}") for b in range(2)]
        a_psum = ps.tile([128, T16, C], f32, tag="a_ps")
        nc.vector.memset(a_psum[:], 0.0)

        for it in range(3):
            last = it == 2
            if it == 0:
                # b_ij = 0 -> cij = 0.1 exactly; write the diagonal directly
                for ms in range(2):
                    nc.vector.memset(
                        cij_bd[ms * 64:ms * 64 + G, :, ms, :].bitcast(f32), 0.1)
            else:
                # ---- softmax over capsules (no max-shift; logits bounded) ----
                eb = sb.tile([128, T16, C], f32, tag="eb")
                ssum = sb.tile([128, T16], f32, tag="ssum")
                rs = sb.tile([128, T16], f32, tag="rs")
                cij = sb.tile([128, T16, C], f32, tag="cij")
                nc.scalar.activation(eb[:], b_ij[:],
                                     mybir.ActivationFunctionType.Exp)
                nc.vector.tensor_reduce(ssum[:], eb[:], axis=mybir.AxisListType.X,
                                        op=mybir.AluOpType.add)
                nc.vector.reciprocal(rs[:], ssum[:])
                nc.vector.tensor_mul(cij[:], eb[:],
                                     rs[:, :, None].broadcast_to((128, T16, C)))
                for ms in range(2):
                    nc.vector.tensor_copy(
                        out=cij_bd[ms * 64:ms * 64 + G, :, ms, :],
                        in_=cij[ms * 64:ms * 64 + G, :, :])

            # ---- q: per (t, h) matmul, K=128 over (m_sub, g) ----
            for h in range(2):
                for t in range(T16):
                    nc.tensor.matmul(q_psum[h][:, t], lhsT=u_qp[:, t, h, :],
                                     rhs=cij_bd[:, t, :, :], start=True, stop=True)
            # transpose 32x32 blocks straight out of PSUM; input AP enumerates
            # (c, m) order: col c*32 + t*2 + ms <- psum offset t*20 + ms*10 + c
            q_T = [sb.tile([128, C, M32], dt_s, tag=f"q_T{h}", name=f"q_T{h}") for h in range(2)]
            for h in range(2):
                src_r = q_psum[h][:].rearrange("p t ms c -> p c (t ms)")
                nc.vector.transpose(out=q_T[h][:], in_=src_r)

            # ---- s: per (c, k) accumulate; out rows = (i4, o) 4x-replicated ----
            for c10 in range(C):
                for k in range(2):
                    nc.tensor.matmul(s_psum[:, c10, :], lhsT=w_s4[k][:, c10, :],
                                     rhs=q_T[k][:, c10, :],
                                     start=(k == 0), stop=(k == 1))

            # ---- squash on [64=(i4,o), c, b] ----
            s2 = sb.tile([128, C, BL], dt_m, tag="s2")
            e1 = sb.tile([128, C, BL], f32, tag="e1")
            root = sb.tile([128, C, BL], f32, tag="root")
            den = sb.tile([128, C, BL], f32, tag="den")
            rcp = sb.tile([128, C, BL], f32, tag="rcp")
            scr = sb.tile([128, C, BL], f32, tag="scr")
            fsc = sb.tile([128, C, BL], f32, tag="fsc")
            v_rep = sb.tile([128, C, BL], f32, tag="v_rep")
            p_sb = [sb.tile([128, C, M32], dt_a, tag=f"p_sb{b}", name=f"p_sb{b}") for b in range(2)]
            CG = [slice(0, 5), slice(5, C)]
            for cg in CG:
                nc.scalar.square(s2[:, cg, :], s_psum[:, cg, :])
                nc.tensor.matmul(mag_ps[:, cg, :], lhsT=ones[:], rhs=s2[:, cg, :],
                                 start=True, stop=True)
                nc.scalar.add(e1[:, cg, :], mag_ps[:, cg, :], 1.0)
                nc.scalar.sqrt(root[:, cg, :], mag_ps[:, cg, :])
                # den = (root + eps) * e1 : eps keeps zero-padded lanes finite
                nc.vector.scalar_tensor_tensor(out=den[:, cg, :],
                                               in0=root[:, cg, :], scalar=1e-30,
                                               in1=e1[:, cg, :],
                                               op0=mybir.AluOpType.add,
                                               op1=mybir.AluOpType.mult)
                nc.vector.reciprocal(rcp[:, cg, :], den[:, cg, :])
                nc.vector.tensor_mul(fsc[:, cg, :], mag_ps[:, cg, :], rcp[:, cg, :])
                nc.vector.tensor_mul(v_rep[:, cg, :], s_psum[:, cg, :], fsc[:, cg, :])
                if last:
                    continue
                for j in range(4):
                    nc.vector.tensor_copy(out=v_bd[j * 32:j * 32 + 16, cg, j, :],
                                          in_=v_rep[j * 32:j * 32 + 16, cg, :])
                for blk in range(2):
                    for c10 in range(C)[cg]:
                        nc.tensor.matmul(p_psum[blk][:, c10, :],
                                         lhsT=v_bd[:, c10, :, :],
                                         rhs=w_p2[:, c10, blk, :],
                                         start=True, stop=True)

            if last:
                nc.sync.dma_start(out=v_out_p[:], in_=v_rep[0:16, :, :])
                break

            for blk in range(2):
                nc.vector.tensor_copy(out=p_sb[blk][:], in_=p_psum[blk][:])

            # ---- a: per (m, k) accumulate into [ (m%2)*64+g , m//2, c ] ----
            for m in range(M32):
                t, ms = m // 2, m % 2
                for k in range(2):
                    nc.tensor.matmul(a_psum[ms * 64:ms * 64 + G, t, :],
                                     lhsT=u_a2[k][:, :, m],
                                     rhs=p_sb[k][:, :, m],
                                     start=(k == 0), stop=(k == 1))
            a_sb = sb.tile([128, T16, C], f32, tag="a_sb")
            a_red = sb.tile([128, T16, C], f32, tag="a_red")
            nc.vector.tensor_copy(out=a_sb[:], in_=a_psum[:])

            cc_in = dr.tile([128, T16 * C], f32, tag=f"cc_in{it}", name=f"cc_in{it}")
            cc_out = dr.tile([128, T16 * C], f32, tag=f"cc_out{it}",
                             name=f"cc_out{it}", addr_space="Shared")
            nc.sync.dma_start(out=cc_in[:], in_=a_sb[:])
            nc.gpsimd.collective_compute(
                "AllReduce", mybir.AluOpType.add, replica_groups=rg,
                ins=[cc_in[:].opt()], outs=[cc_out[:].opt()])
            nc.sync.dma_start(out=a_red[:], in_=cc_out[:])
            nc.vector.tensor_add(b_ij[:], b_ij[:], a_red[:])

    nc.finalize()
    return nc


_NC_CACHE = None


def kernel(u, W):
    """u [256,1152,8] f32, W [32,10,16,8] f32 -> [256,10,16,1] f32."""
    global _NC_CACHE
    from concourse import bass_utils

    in_maps = _host_prep(u, W)
    if _NC_CACHE is None:
        _NC_CACHE = _build_nc()
    res = bass_utils.run_bass_kernel_spmd(
        _NC_CACHE, in_maps, core_ids=list(range(NC_CORES)))

    out = np.zeros((NC_CORES * BL, C, O, 1), np.float32)
    for c in range(NC_CORES):
        vo = res.results[c]["v_out"]          # [16, C, BL] = [o, c, b]
        out[c * BL:(c + 1) * BL, :, :, 0] = vo.transpose(2, 1, 0)
    return out


if __name__ == "__main__":
    u = np.random.randn(256, 1152, 8).astype(np.float32)
    W = np.random.randn(32, 10, 16, 8).astype(np.float32)
    v = kernel(u, W)
    print("kernel ran, out shape", v.shape, "absmax", np.abs(v).max())



# revision 1
# speedup vs baseline: 1.0125x; 1.0125x over previous
"""Trainium2 Bass kernel for DigitCapsuleLayer dynamic routing.

Strategy: data-parallel over batch (32 per core x 8 cores). The routing is
computed in a fully factored form that never materializes u_hat
[B,1152,10,16]:

  q[b,c,m,i] = sum_g  cij[(g,m),c] * u[b,(g,m),i]      (PE, block-diag cij)
  s[b,c,o]   = sum_mi W[m,c,o,i]   * q[b,c,m,i]        (PE, after a DVE
                                                        32x32 block transpose
                                                        moves i to partitions)
  v = squash(s)                                        (PE ones-trick + DVE/ACT)
  p[b,c,m,i] = sum_o  W[m,c,o,i]   * v[b,c,o]          (PE, block-diag v)
  a[r,c]     = sum_bi u[b,r,i]/B   * p[b,c,m,i]        (PE)
  AllReduce(a) across 8 cores; b_ij += a; cij = softmax(b_ij)

Indices: r = g*32+m (g<36, m<32), m = 2t+m_sub, i = h*4+i4 (h<2, i4<4).
Partition layouts:  P1 rows = m_sub*64+g (rows 36..63/100..127 zero-padded),
q/p rows = i4*32+b (per half/blk), s/v rows = i4*16+o (4x replicated).
All index algebra validated against the jax reference in proto.py.
"""

import os
import sys
import numpy as np

sys.path.insert(0, "/opt/trn_rl_repo")
sys.path.insert(0, "/opt/trn_rl_repo/concourse")

NC_CORES = 8
BL = 32           # batch per core
G, M32, C, O, I = 36, 32, 10, 16, 8
T16 = 16
F32 = None        # set after mybir import


# ----------------------------------------------------------------- host prep
def _host_prep(u, W):
    """u [256,1152,8] f32, W [32,10,16,8] f32 -> per-core input maps."""
    u = np.ascontiguousarray(u, np.float32)
    W = np.ascontiguousarray(W, np.float32)

    # u_qp [core, 128, t, h, 128]; row p = m_sub*64+g ; col = i4*32+b
    u8 = u.reshape(NC_CORES, BL, G, T16, 2, 2, 4)   # [n, b, g, t, ms, h, i4]
    perm = u8.transpose(0, 2, 4, 3, 5, 6, 1)        # [n, g, ms, t, h, i4, b]
    u_qp = np.zeros((NC_CORES, 128, T16, 2, 128), np.float32)
    u_qp_v = u_qp.reshape(NC_CORES, 128, T16, 2, 4, 32)
    for ms in range(2):
        u_qp_v[:, ms * 64:ms * 64 + G] = perm[:, :, ms]

    # u_a2 [core, k, 128, 36, 32]: row i4*32+b, col (g, m); prescaled by 1/256
    ua = u.reshape(NC_CORES, BL, G, M32, 2, 4)      # [n, b, g, m, k, i4]
    u_a2 = np.ascontiguousarray(
        ua.transpose(0, 4, 5, 1, 2, 3), np.float32
    ).reshape(NC_CORES, 2, 128, G, M32) * np.float32(1.0 / 256.0)

    # w_s4 [k, 128, c, 128]: row i4*32+m, col (rep, half, o): half0 = W[m,c,o,i],
    # half1 = 0 pad so matmul M=128 lands v at partitions rep*32+o.
    wi = W.transpose(3, 0, 1, 2)                    # [i, m, c, o]
    w_s4 = np.zeros((2, 4, M32, C, 4, 2, O), np.float32)
    w_s4[:, :, :, :, :, 0, :] = wi.reshape(2, 4, M32, C, 1, O)
    w_s4 = np.ascontiguousarray(w_s4.reshape(2, 128, C, 128))

    # w_p2 [128, c, blk, m]: row i4*32+o (o<16; +16..31 zero) = W[m,c,o,blk*4+i4]
    wp = W.transpose(3, 2, 1, 0)                    # [i, o, c, m]
    w_p2 = np.zeros((4, 2, O, C, 2, M32), np.float32)
    w_p2[:, 0] = wp.reshape(2, 4, O, C, M32).transpose(1, 2, 3, 0, 4)
    w_p2 = np.ascontiguousarray(w_p2.reshape(128, C, 2, M32))

    # ones_bd [128, 128]: 16x16 ones blocks at (i4*32..+16, i4*32..+16)
    ones_bd = np.zeros((4, 2, O, 4, 2, O), np.float32)
    for j in range(4):
        ones_bd[j, 0, :, j, 0, :] = 1.0
    ones_bd = np.ascontiguousarray(ones_bd.reshape(128, 128))

    in_maps = []
    for c in range(NC_CORES):
        in_maps.append({
            "u_qp": u_qp[c],
            "u_a2": u_a2[c],
            "w_s4": w_s4,
            "w_p2": w_p2,
            "ones_bd": ones_bd,
        })
    return in_maps


# ------------------------------------------------------------- bass builder
def _build_nc():
    from contextlib import ExitStack
    import concourse.bacc as bacc
    import concourse.tile as tile
    from concourse import mybir

    f32 = mybir.dt.float32
    f32r = mybir.dt.float32r
    # per-matmul-family fp32r enable (bisection switches)
    RQ = RS = RM = RP = RA = False
    dt_q = f32r if RQ else f32
    dt_s = f32r if RS else f32
    dt_m = f32r if RM else f32
    dt_p = f32r if RP else f32
    dt_a = f32r if RA else f32
    nc = bacc.Bacc("TRN2", target_bir_lowering=False, debug=False,
                   num_devices=NC_CORES)

    u_qp_p = nc.dram_tensor("u_qp", [128, T16, 2, 128], dt_q, kind="ExternalInput")
    u_a2_p = nc.dram_tensor("u_a2", [2, 128, G, M32], dt_a, kind="ExternalInput")
    w_s4_p = nc.dram_tensor("w_s4", [2, 128, C, 128], dt_s, kind="ExternalInput")
    w_p2_p = nc.dram_tensor("w_p2", [128, C, 2, M32], dt_p, kind="ExternalInput")
    ones_p = nc.dram_tensor("ones_bd", [128, 128], dt_m, kind="ExternalInput")
    v_out_p = nc.dram_tensor("v_out", [16, C, BL], f32, kind="ExternalOutput")

    rg = [list(range(NC_CORES))]

    with tile.TileContext(nc) as tc, ExitStack() as ctx:
        sb = ctx.enter_context(tc.tile_pool(name="sb", bufs=1))
        ps = ctx.enter_context(tc.tile_pool(name="ps", bufs=1, space="PSUM"))
        dr = ctx.enter_context(tc.tile_pool(name="dr", bufs=1, space="DRAM"))

        # --- persistent SBUF tiles ---
        u_qp = sb.tile([128, T16, 2, 128], dt_q, tag="u_qp")
        u_a2 = [sb.tile([128, G, M32], dt_a, tag=f"u_a2_{k}", name=f"u_a2_{k}") for k in range(2)]
        w_s4 = [sb.tile([128, C, 128], dt_s, tag=f"w_s4_{k}", name=f"w_s4_{k}") for k in range(2)]
        w_p2 = sb.tile([128, C, 2, M32], dt_p, tag="w_p2")
        ones = sb.tile([128, 128], dt_m, tag="ones")
        b_ij = sb.tile([128, T16, C], f32, tag="b_ij")
        cij_bd = sb.tile([128, T16, 2, C], dt_q, tag="cij_bd")
        v_bd = sb.tile([128, C, 4, BL], dt_p, tag="v_bd")

        for tchunk in range(4):
            nc.sync.dma_start(out=u_qp[:, 4 * tchunk:4 * (tchunk + 1)],
                              in_=u_qp_p[:, 4 * tchunk:4 * (tchunk + 1)])
        for k in range(2):
            nc.sync.dma_start(out=u_a2[k][:], in_=u_a2_p[k])
            nc.sync.dma_start(out=w_s4[k][:], in_=w_s4_p[k])
        nc.sync.dma_start(out=w_p2[:], in_=w_p2_p[:])
        nc.sync.dma_start(out=ones[:], in_=ones_p[:])
        nc.vector.memset(b_ij[:], 0.0)
        # warm-up collective: absorbs first-AR setup/handshake off the
        # critical path while iter-0 compute proceeds
        warm_in = dr.tile([1, 20], f32, tag="warm_in", name="warm_in")
        warm_out = dr.tile([1, 20], f32, tag="warm_out", name="warm_out",
                           addr_space="Shared")
        nc.sync.dma_start(out=warm_in[:], in_=b_ij[0:1, 0:2, :])
        nc.gpsimd.collective_compute(
            "AllReduce", mybir.AluOpType.add, replica_groups=rg,
            ins=[warm_in[:].opt()], outs=[warm_out[:].opt()])
        nc.vector.memset(cij_bd[:].bitcast(f32), 0.0)
        nc.vector.memset(v_bd[:].bitcast(f32), 0.0)

        # --- PSUM tiles (persist across iterations) ---
        q_psum = [ps.tile([128, T16, 2, C], f32, tag=f"q_ps{h}", name=f"q_ps{h}") for h in range(2)]
        s_psum = ps.tile([128, C, BL], f32, tag="s_ps")
        mag_ps = ps.tile([128, C, BL], f32, tag="mag_ps")
        p_psum = [ps.tile([128, C, M32], f32, tag=f"p_ps{b}", name=f"p_ps{b}") for b in range(2)]
        a_psum = ps.tile([128, T16, C], f32, tag="a_ps")
        nc.vector.memset(a_psum[:], 0.0)

        for it in range(3):
            last = it == 2
            if it == 0:
                # b_ij = 0 -> cij = 0.1 exactly; write the diagonal directly
                for ms in range(2):
                    nc.vector.memset(
                        cij_bd[ms * 64:ms * 64 + G, :, ms, :].bitcast(f32), 0.1)
            else:
                # ---- softmax over capsules (no max-shift; logits bounded) ----
                eb = sb.tile([128, T16, C], f32, tag="eb")
                ssum = sb.tile([128, T16], f32, tag="ssum")
                rs = sb.tile([128, T16], f32, tag="rs")
                cij = sb.tile([128, T16, C], f32, tag="cij")
                nc.scalar.activation(eb[:], b_ij[:],
                                     mybir.ActivationFunctionType.Exp)
                nc.vector.tensor_reduce(ssum[:], eb[:], axis=mybir.AxisListType.X,
                                        op=mybir.AluOpType.add)
                nc.vector.reciprocal(rs[:], ssum[:])
                nc.vector.tensor_mul(cij[:], eb[:],
                                     rs[:, :, None].broadcast_to((128, T16, C)))
                for ms in range(2):
                    nc.vector.tensor_copy(
                        out=cij_bd[ms * 64:ms * 64 + G, :, ms, :],
                        in_=cij[ms * 64:ms * 64 + G, :, :])

            # ---- q: per (t, h) matmul, K=128 over (m_sub, g) ----
            for h in range(2):
                for t in range(T16):
                    nc.tensor.matmul(q_psum[h][:, t], lhsT=u_qp[:, t, h, :],
                                     rhs=cij_bd[:, t, :, :], start=True, stop=True)
            # transpose 32x32 blocks straight out of PSUM; input AP enumerates
            # (c, m) order: col c*32 + t*2 + ms <- psum offset t*20 + ms*10 + c
            q_T = [sb.tile([128, C, M32], dt_s, tag=f"q_T{h}", name=f"q_T{h}") for h in range(2)]
            for h in range(2):
                src_r = q_psum[h][:].rearrange("p t ms c -> p c (t ms)")
                nc.vector.transpose(out=q_T[h][:], in_=src_r)

            # ---- s: per (c, k) accumulate; out rows = (i4, o) 4x-replicated ----
            for c10 in range(C):
                for k in range(2):
                    nc.tensor.matmul(s_psum[:, c10, :], lhsT=w_s4[k][:, c10, :],
                                     rhs=q_T[k][:, c10, :],
                                     start=(k == 0), stop=(k == 1))

            # ---- squash on [64=(i4,o), c, b] ----
            s2 = sb.tile([128, C, BL], dt_m, tag="s2")
            e1 = sb.tile([128, C, BL], f32, tag="e1")
            root = sb.tile([128, C, BL], f32, tag="root")
            den = sb.tile([128, C, BL], f32, tag="den")
            rcp = sb.tile([128, C, BL], f32, tag="rcp")
            scr = sb.tile([128, C, BL], f32, tag="scr")
            fsc = sb.tile([128, C, BL], f32, tag="fsc")
            v_rep = sb.tile([128, C, BL], f32, tag="v_rep")
            p_sb = [sb.tile([128, C, M32], dt_a, tag=f"p_sb{b}", name=f"p_sb{b}") for b in range(2)]
            CG = [slice(0, 5), slice(5, C)]
            for cg in CG:
                nc.scalar.square(s2[:, cg, :], s_psum[:, cg, :])
                nc.tensor.matmul(mag_ps[:, cg, :], lhsT=ones[:], rhs=s2[:, cg, :],
                                 start=True, stop=True)
                nc.scalar.add(e1[:, cg, :], mag_ps[:, cg, :], 1.0)
                nc.scalar.sqrt(root[:, cg, :], mag_ps[:, cg, :])
                # den = (root + eps) * e1 : eps keeps zero-padded lanes finite
                nc.vector.scalar_tensor_tensor(out=den[:, cg, :],
                                               in0=root[:, cg, :], scalar=1e-30,
                                               in1=e1[:, cg, :],
                                               op0=mybir.AluOpType.add,
                                               op1=mybir.AluOpType.mult)
                nc.vector.reciprocal(rcp[:, cg, :], den[:, cg, :])
                nc.vector.tensor_mul(fsc[:, cg, :], mag_ps[:, cg, :], rcp[:, cg, :])
                nc.vector.tensor_mul(v_rep[:, cg, :], s_psum[:, cg, :], fsc[:, cg, :])
                if last:
                    continue
                for j in range(4):
                    nc.vector.tensor_copy(out=v_bd[j * 32:j * 32 + 16, cg, j, :],
                                          in_=v_rep[j * 32:j * 32 + 16, cg, :])
                for blk in range(2):
                    for c10 in range(C)[cg]:
                        nc.tensor.matmul(p_psum[blk][:, c10, :],
                                         lhsT=v_bd[:, c10, :, :],
                                         rhs=w_p2[:, c10, blk, :],
                                         start=True, stop=True)

            if last:
                nc.sync.dma_start(out=v_out_p[:], in_=v_rep[0:16, :, :])
                break

            for blk in range(2):
                nc.vector.tensor_copy(out=p_sb[blk][:], in_=p_psum[blk][:])

            # ---- a: per (m, k) accumulate into [ (m%2)*64+g , m//2, c ] ----
            for m in range(M32):
                t, ms = m // 2, m % 2
                for k in range(2):
                    nc.tensor.matmul(a_psum[ms * 64:ms * 64 + G, t, :],
                                     lhsT=u_a2[k][:, :, m],
                                     rhs=p_sb[k][:, :, m],
                                     start=(k == 0), stop=(k == 1))
            a_sb = sb.tile([128, T16, C], f32, tag="a_sb")
            a_red = sb.tile([128, T16, C], f32, tag="a_red")
            nc.vector.tensor_copy(out=a_sb[:], in_=a_psum[:])

            cc_in = dr.tile([128, T16 * C], f32, tag=f"cc_in{it}", name=f"cc_in{it}")
            cc_out = dr.tile([128, T16 * C], f32, tag=f"cc_out{it}",
                             name=f"cc_out{it}", addr_space="Shared")
            nc.sync.dma_start(out=cc_in[:], in_=a_sb[:])
            nc.gpsimd.collective_compute(
                "AllReduce", mybir.AluOpType.add, replica_groups=rg,
                ins=[cc_in[:].opt()], outs=[cc_out[:].opt()])
            nc.sync.dma_start(out=a_red[:], in_=cc_out[:])
            nc.vector.tensor_add(b_ij[:], b_ij[:], a_red[:])

    nc.finalize()
    return nc


_NC_CACHE = None


def kernel(u, W):
    """u [256,1152,8] f32, W [32,10,16,8] f32 -> [256,10,16,1] f32."""
    global _NC_CACHE
    from concourse import bass_utils

    in_maps = _host_prep(u, W)
    if _NC_CACHE is None:
        _NC_CACHE = _build_nc()
    res = bass_utils.run_bass_kernel_spmd(
        _NC_CACHE, in_maps, core_ids=list(range(NC_CORES)))

    out = np.zeros((NC_CORES * BL, C, O, 1), np.float32)
    for c in range(NC_CORES):
        vo = res.results[c]["v_out"]          # [16, C, BL] = [o, c, b]
        out[c * BL:(c + 1) * BL, :, :, 0] = vo.transpose(2, 1, 0)
    return out


if __name__ == "__main__":
    u = np.random.randn(256, 1152, 8).astype(np.float32)
    W = np.random.randn(32, 10, 16, 8).astype(np.float32)
    v = kernel(u, W)
    print("kernel ran, out shape", v.shape, "absmax", np.abs(v).max())

